# revision 2
# baseline (speedup 1.0000x reference)
"""GAT (graph attention) layer on 8 Trainium2 NeuronCores — v2.

Strategy (dst-partitioned edge parallel, bulk-gather based):
  - Nodes are split into 8 contiguous ranges of 6250; each core's nodes are
    permuted into NW=51 "windows" of <=127 nodes (position 127 = trash slot),
    so a node's permuted row is (core, window, position).
  - Phase A (per core): project the core's node shard with TensorE in bf16:
        [Wh | ed] = h_shard @ [Wmat | A_d] + bias
    where Wh is the per-head projection in (o,h)-interleaved layout (256 cols)
    and ed[h] = <Wh row, a_dst[h]> + c_d + ab (destination attention score).
    Wh rows (512B each) go to a DRAM table that is AllGathered so every core
    holds all 52224 rows; ed rows (padded to 256B) stay in a core-local table.
  - Phase B (per core): edges are grouped by destination window (host-side),
    each window holding <=CH*128 edges per src-half (src row < or >= 26112,
    so bulk dma_gather int16 indices stay in range).  Per window:
      * one bulk dma_gather per src-half: G[128, CW, 256] = Wh[src] rows
      * e_s per edge on DVE: reduce_o(G * a_src)     (a_src broadcast mult)
      * e_d per edge: bulk dma_gather from the local ed table by dst position
      * w = max(exp(s), exp(alpha*s)) = exp(leakyrelu(s))   (ACT + DVE max)
      * one-hot A[e,v] = (iota - dstl == 0) per 128-edge chunk (DVE)
      * numerator+denominator via PSUM-accumulated matmuls:
            agg[v, :] += A_k.T @ [w*G | w]_k
      * out = agg[:, :256] / max(agg[:, 256:], tiny)
  - Host does only index/layout work: window balancing, packing, unscramble.
"""
import os
import sys

sys.path.insert(0, "/opt/trn_rl_repo")

import numpy as np
import ml_dtypes

import concourse.bass as bass
import concourse.bacc as bacc
import concourse.tile as tile
import concourse.mybir as mybir

BF16 = np.dtype(ml_dtypes.bfloat16)
P = 128

FULL_CFG = dict(
    N=50000, F=512, H=8, O=32, ALPHA=0.2, NCORES=8, NW=51,
)

_LAST_RESULTS = {}


# --------------------------------------------------------------------------
# Host-side planning
# --------------------------------------------------------------------------

def _plan(cfg, src, dst):
    """Balance each core's nodes into windows; pack edges into chunk slots."""
    import heapq

    N, NCORES, NW = cfg["N"], cfg["NCORES"], cfg["NW"]
    NPC = N // NCORES              # nodes per core
    NS = NW * P                    # padded rows per core
    E = src.shape[0]

    core_of = (np.arange(N) // NPC).astype(np.int64)
    src_half = (src // NPC) < (NCORES // 2)   # True -> table A

    degA = np.zeros(N, np.int64)
    degB = np.zeros(N, np.int64)
    np.add.at(degA, dst[src_half], 1)
    np.add.at(degB, dst[~src_half], 1)

    slot_of = np.empty(N, np.int32)
    pos_of = np.empty(N, np.int32)
    max_half_edges = 0
    for c in range(NCORES):
        lo, hi = NPC * c, NPC * (c + 1)
        dA, dB = degA[lo:hi], degB[lo:hi]
        order = np.argsort(-(dA + dB), kind="stable")
        la = np.zeros(NW)
        lb = np.zeros(NW)
        cnt = np.zeros(NW, np.int64)
        heap = [(0.0, 0.0, 0, w) for w in range(NW)]
        heapq.heapify(heap)
        for i in order:
            tmp = []
            while True:
                item = heapq.heappop(heap)
                w = item[3]
                if cnt[w] < P - 1:
                    break
                tmp.append(item)
            for t in tmp:
                heapq.heappush(heap, t)
            slot_of[lo + i] = w
            pos_of[lo + i] = cnt[w]
            la[w] += dA[i]
            lb[w] += dB[i]
            cnt[w] += 1
            if cnt[w] < P - 1:
                heapq.heappush(heap, (max(la[w], lb[w]), la[w] + lb[w],
                                      int(cnt[w]), w))
        max_half_edges = max(max_half_edges, int(la.max()), int(lb.max()))

    CH = max(1, -(-max_half_edges // P))   # chunks per half-window
    CW = 2 * CH
    cap = CH * P

    # permuted rows
    perm_row = core_of * NS + slot_of.astype(np.int64) * P + pos_of
    HALF_ROWS = (NCORES // 2) * NS

    # per-core packed arrays
    srcidx = np.zeros((NCORES, 2, P, NW * CH), np.int32)   # [core, half, p, col]
    dstl = np.full((NCORES, P, NW * CW), 127.0, np.float32)

    dst_core = core_of[dst]
    for c in range(NCORES):
        m = dst_core == c
        s_c, d_c = src[m], dst[m]
        half_c = (~src_half[m]).astype(np.int64)   # 0 -> A, 1 -> B
        w_c = slot_of[d_c].astype(np.int64)
        key = w_c * 2 + half_c
        order = np.argsort(key, kind="stable")
        s_c, d_c, half_c, w_c = (s_c[order], d_c[order], half_c[order],
                                 w_c[order])
        key = key[order]
        counts = np.bincount(key, minlength=NW * 2)
        assert counts.max() <= cap, (counts.max(), cap)
        off = np.concatenate([[0], np.cumsum(counts)[:-1]])
        within = np.arange(len(s_c)) - off[key]
        p = within % P
        k = within // P                      # chunk within the half
        # src gather index (relative to its half table)
        g = perm_row[s_c] - half_c * HALF_ROWS
        if len(g):
            assert g.min() >= 0 and g.max() < HALF_ROWS
        srcidx[c, half_c, p, w_c * CH + k] = g
        # one-hot position (dst is local to core c)
        col = w_c * CW + half_c * CH + k
        dstl[c, p, col] = pos_of[d_c]

    # one-hots streamed to the device, per chunk column col and edge slot e:
    #   atm[core][v, col*P + e] = 1 iff dstl[core][e, col] == v   (AT[v, e])
    #   am [core][e, col*P + v] = 1 iff dstl[core][e, col] == v   (A[e, v])
    FP8 = np.dtype(ml_dtypes.float8_e4m3)
    atm = np.zeros((NCORES, P, NW * CW * P), FP8)
    am = np.zeros((NCORES, P, NW * CW * P), FP8)
    cols = np.arange(NW * CW)[None, :].repeat(P, 0)
    e_ix = np.arange(P)[:, None].repeat(NW * CW, 1)
    for c in range(NCORES):
        pos = dstl[c].astype(np.int64)            # [P(e), COLS]
        atm[c, pos.reshape(-1), (cols * P + e_ix).reshape(-1)] = 1.0
        am[c, e_ix.reshape(-1), (cols * P + pos).reshape(-1)] = 1.0

    return dict(CH=CH, NS=NS, HALF_ROWS=HALF_ROWS,
                srcidx=srcidx, dstl=dstl, atm=atm, am=am,
                slot_of=slot_of, pos_of=pos_of)


def _pack_idx(idx_pcol, cols_per_window):
    """[P, COLS] int32 slot-layout -> dma_gather int16 idx layout [128, COLS*8].

    Per window, chunk-columns are gathered in calls of <=8 columns (1024
    indices).  Within a call, flat index i = k*128 + p maps to idx tile
    [i % 16, i // 16]; call data is packed contiguously so the call whose
    first chunk-column is c starts at idx column c*8.
    """
    Ptot, COLS = idx_pcol.shape
    assert Ptot == P
    cpw = cols_per_window
    assert COLS % cpw == 0
    out = np.empty((16, COLS * 8), np.int16)
    pos = 0
    for w in range(COLS // cpw):
        for base in range(0, cpw, 8):
            nb = min(8, cpw - base)
            nidx = nb * P
            flat = idx_pcol[:, w * cpw + base:w * cpw + base + nb].T.reshape(-1)
            out[:, pos:pos + nidx // 16] = flat.reshape(nidx // 16, 16).T
            pos += nidx // 16
    assert pos == COLS * 8
    return np.tile(out, (8, 1))


def _host_weights(cfg, W, Wb, a, ab):
    H, F, O = W.shape
    a_src, a_dst = a[:, :O], a[:, O:]
    Wmat = W.transpose(1, 0, 2).reshape(F, H * O)          # [F, (h,o)]
    A_d = np.einsum("hfo,ho->fh", W, a_dst)
    A_s = np.einsum("hfo,ho->fh", W, a_src)
    wext = np.concatenate([Wmat, A_d, A_s], axis=1)        # [F, 272]
    c_d = (Wb * a_dst).sum(1)
    c_s = (Wb * a_src).sum(1)
    bext = np.concatenate([Wb.reshape(-1), c_d + ab,
                           c_s]).astype(np.float32)
    return wext.astype(np.float32), bext


# --------------------------------------------------------------------------
# Device program
# --------------------------------------------------------------------------

def build_gat(cfg):
    N, F, H, O, NCORES = cfg["N"], cfg["F"], cfg["H"], cfg["O"], cfg["NCORES"]
    NW, CH, NS = cfg["NW"], cfg["CH"], cfg["NS"]
    CW = 2 * CH
    HO = H * O                    # 256
    AD = HO + H                   # 264
    NT = NW                       # phase-A tiles per core
    KT = F // P
    NSG = NS * NCORES
    HALF_ROWS = cfg["HALF_ROWS"]
    RW = 384                      # gather row: Wh(256) | es(8) | pad (768B)
    AD2 = HO + 2 * H              # phase-A psum width: Wh | ed | es

    bf = mybir.dt.bfloat16
    f32 = mybir.dt.float32

    nc = bacc.Bacc("TRN2", target_bir_lowering=False, debug=False,
                   num_devices=NCORES, num_swdge_queues=4)

    hT = nc.dram_tensor("hT", [F, NS], bf, kind="ExternalInput")
    wext = nc.dram_tensor("wext", [F, AD2], bf, kind="ExternalInput")
    bext = nc.dram_tensor("bext", [1, AD2], bf, kind="ExternalInput")
    ones1 = nc.dram_tensor("ones1", [1, P], bf, kind="ExternalInput")
    idxA = nc.dram_tensor("idxA", [P, NW * CH * 8], mybir.dt.int16,
                          kind="ExternalInput")
    idxB = nc.dram_tensor("idxB", [P, NW * CH * 8], mybir.dt.int16,
                          kind="ExternalInput")
    atm = nc.dram_tensor("atm", [P, NW * CW * P], mybir.dt.float8e4,
                         kind="ExternalInput")
    am = nc.dram_tensor("am", [P, NW * CW * P], mybir.dt.float8e4,
                        kind="ExternalInput")

    out_local = nc.dram_tensor("out_local", [NS, HO], f32,
                               kind="ExternalOutput")

    with tile.TileContext(nc) as tc:
        with (
            tc.tile_pool(name="dram", bufs=1, space="DRAM") as dram,
            tc.tile_pool(name="const", bufs=1) as cpool,
        ):
            tbl_local = dram.tile([NS, RW], bf)
            tbl_global = dram.tile([NSG, RW], bf, addr_space="Shared")

            ones_t = cpool.tile([1, P], bf)
            nc.sync.dma_start(out=ones_t[:], in_=ones1[:, :])
            bext_t = cpool.tile([1, AD2], bf)
            nc.sync.dma_start(out=bext_t[:], in_=bext[:, :])
            idxA_t = cpool.tile([P, NW * CH * 8], mybir.dt.int16)
            nc.sync.dma_start(out=idxA_t[:], in_=idxA[:, :])
            idxB_t = cpool.tile([P, NW * CH * 8], mybir.dt.int16)
            nc.sync.dma_start(out=idxB_t[:], in_=idxB[:, :])
            ed_sb = cpool.tile([P, NW * H], bf)

            # ---------------- Phase A: projection ----------------
            with (
                tc.tile_pool(name="pa_sb", bufs=1) as pa,
                tc.tile_pool(name="pa_ps", bufs=2, space="PSUM") as pa_ps,
            ):
                hT_t = pa.tile([P, KT * NS], bf, tag="hT")
                for kk in range(KT):
                    nc.sync.dma_start(out=hT_t[:, kk * NS:(kk + 1) * NS],
                                      in_=hT[kk * P:(kk + 1) * P, :])
                wext_t = pa.tile([P, KT * AD2], bf, tag="wext")
                for kk in range(KT):
                    nc.sync.dma_start(out=wext_t[:, kk * AD2:(kk + 1) * AD2],
                                      in_=wext[kk * P:(kk + 1) * P, :])

                stage = pa.tile([P, NT * RW], bf, tag="stage")
                nc.vector.memset(stage[:], 0.0)

                for t in range(NT):
                    psA = pa_ps.tile([P, AD2], f32, tag="psA")
                    for kk in range(KT):
                        nc.tensor.matmul(
                            out=psA[:],
                            lhsT=hT_t[:, kk * NS + t * P: kk * NS + (t + 1) * P],
                            rhs=wext_t[:, kk * AD2:(kk + 1) * AD2],
                            start=(kk == 0), stop=False)
                    nc.tensor.matmul(out=psA[:], lhsT=ones_t[:],
                                     rhs=bext_t[:], start=False, stop=True)
                    nc.vector.tensor_copy(
                        out=stage[:, t * RW:t * RW + HO], in_=psA[:, 0:HO])
                    nc.vector.tensor_copy(
                        out=stage[:, t * RW + HO:t * RW + HO + H],
                        in_=psA[:, HO + H:HO + 2 * H])
                    nc.vector.tensor_copy(
                        out=ed_sb[:, t * H:(t + 1) * H],
                        in_=psA[:, HO:HO + H])

                nc.sync.dma_start(
                    out=tbl_local[:].rearrange("(t p) d -> p t d", p=P),
                    in_=stage[:].rearrange("p (t d) -> p t d", t=NT))

            nc.gpsimd.collective_compute(
                "AllGather",
                mybir.AluOpType.bypass,
                replica_groups=[list(range(NCORES))],
                ins=[tbl_local.opt()],
                outs=[tbl_global.opt()],
            )

            # ---------------- Phase B: edges ----------------
            with (
                tc.tile_pool(name="g_sb", bufs=6) as gpool,
                tc.tile_pool(name="at_sb", bufs=4) as atpool,
                tc.tile_pool(name="s_sb", bufs=3) as spool,
                tc.tile_pool(name="a_sb", bufs=4) as apool,
                tc.tile_pool(name="gp_sb", bufs=3) as gppool,
                tc.tile_pool(name="o_sb", bufs=2) as opool,
                tc.tile_pool(name="agg_ps", bufs=2, space="PSUM") as aggp,
                tc.tile_pool(name="pse_ps", bufs=2, space="PSUM") as psep,
            ):
                viewA = tbl_global[0:HALF_ROWS, :]
                viewB = tbl_global[HALF_ROWS:2 * HALF_ROWS, :]
                for w in range(NW):
                    G = gpool.tile([P, CW * RW], bf, tag="g")
                    for half, view, it in ((0, viewA, idxA_t),
                                           (1, viewB, idxB_t)):
                        for base in range(0, CH, 8):
                            nb = min(8, CH - base)
                            nidx = nb * P
                            c0 = half * CH + base
                            nc.gpsimd.dma_gather(
                                G[:, c0 * RW:(c0 + nb) * RW].rearrange(
                                    "p (k e) -> p k e", k=nb),
                                view,
                                it[:, (w * CH + base) * 8:
                                   (w * CH + base) * 8 + nidx // 16],
                                nidx, nidx, RW,
                                queue_num=(w * 2 + half) % 4)
                    # e_d per edge: streamed transposed one-hot + matmul
                    at_t = atpool.tile([P, CW * P], mybir.dt.float8e4,
                                       tag="at")
                    nc.sync.dma_start(
                        out=at_t[:], in_=atm[:, w * CW * P:(w + 1) * CW * P])
                    psE = psep.tile([P, CW * H], f32, tag="psE")
                    for k in range(CW):
                        nc.tensor.matmul(
                            out=psE[:, k * H:(k + 1) * H],
                            lhsT=at_t[:, k * P:(k + 1) * P],
                            rhs=ed_sb[:, w * H:(w + 1) * H],
                            start=True, stop=True)

                    # s = es (gathered) + ed
                    s_t = spool.tile([P, CW * H], f32, tag="s")
                    nc.vector.tensor_tensor(
                        out=s_t[:].rearrange("p (k h) -> p k h", k=CW),
                        in0=G[:].rearrange("p (k r) -> p k r",
                                           k=CW)[:, :, HO:HO + H],
                        in1=psE[:].rearrange("p (k h) -> p k h", k=CW),
                        op=mybir.AluOpType.add)

                    # w = max(exp(s), exp(alpha*s))
                    w1 = spool.tile([P, CW * H], f32, tag="w1")
                    nc.scalar.activation(out=w1[:], in_=s_t[:],
                                         func=mybir.ActivationFunctionType.Exp)
                    w2 = spool.tile([P, CW * H], f32, tag="w2")
                    nc.scalar.activation(out=w2[:], in_=s_t[:],
                                         func=mybir.ActivationFunctionType.Exp,
                                         scale=float(cfg["ALPHA"]))
                    wv = spool.tile([P, CW * H], f32, tag="wv")
                    nc.vector.tensor_tensor(out=wv[:], in0=w1[:], in1=w2[:],
                                            op=mybir.AluOpType.max)

                    # gp = [w * G | w]
                    gp = gppool.tile([P, CW * AD], bf, tag="gp")
                    gp4 = gp[:].rearrange("p (k d) -> p k d", k=CW)
                    nc.vector.tensor_tensor(
                        out=gp4[:, :, 0:HO].rearrange(
                            "p k (h o) -> p k h o", h=H),
                        in0=G[:].rearrange("p (k r) -> p k r",
                                           k=CW)[:, :, 0:HO].rearrange(
                            "p k (h o) -> p k h o", h=H),
                        in1=wv[:].rearrange("p (k h) -> p k h",
                                            k=CW)[:, :, :, None].to_broadcast(
                            [P, CW, H, O]),
                        op=mybir.AluOpType.mult)
                    nc.scalar.activation(
                        out=gp4[:, :, HO:AD],
                        in_=wv[:].rearrange("p (k h) -> p k h", k=CW),
                        func=mybir.ActivationFunctionType.Copy)

                    # streamed one-hot + aggregation
                    a_t = apool.tile([P, CW * P], mybir.dt.float8e4,
                                     tag="a")
                    nc.sync.dma_start(
                        out=a_t[:], in_=am[:, w * CW * P:(w + 1) * CW * P])
                    agg = aggp.tile([P, AD], f32, tag="agg")
                    for k in range(CW):
                        nc.tensor.matmul(
                            out=agg[:],
                            lhsT=a_t[:, k * P:(k + 1) * P],
                            rhs=gp[:, k * AD:(k + 1) * AD],
                            start=(k == 0), stop=(k == CW - 1))

                    den = opool.tile([P, H], f32, tag="den")
                    nc.vector.tensor_scalar(
                        out=den[:], in0=agg[:, HO:AD],
                        scalar1=1e-30, scalar2=None,
                        op0=mybir.AluOpType.max)
                    rec = opool.tile([P, H], f32, tag="rec")
                    nc.vector.reciprocal(out=rec[:], in_=den[:])
                    o_t = opool.tile([P, HO], f32, tag="o")
                    nc.vector.tensor_tensor(
                        out=o_t[:].rearrange("p (h o) -> p h o", h=H),
                        in0=agg[:, 0:HO].rearrange("p (h o) -> p h o", h=H),
                        in1=rec[:][:, :, None].to_broadcast([P, H, O]),
                        op=mybir.AluOpType.mult)
                    nc.sync.dma_start(
                        out=out_local[w * P:(w + 1) * P, :], in_=o_t[:])

    return nc


# --------------------------------------------------------------------------
# Execution (PJRT path with steady-state timing)
# --------------------------------------------------------------------------

def _run_pjrt_timed(nc, in_maps, n_cores, n_reps=1):
    import time

    import jax
    from jax.sharding import Mesh, PartitionSpec
    from jax.experimental.shard_map import shard_map

    from concourse import bass2jax
    from concourse import mybir as mb

    bass2jax.install_neuronx_cc_hook()

    partition_name = (nc.partition_id_tensor.name
                      if nc.partition_id_tensor else None)

    in_names, out_names, out_avals, zero_outs = [], [], [], []
    for alloc in nc.m.functions[0].allocations:
        if not isinstance(alloc, mb.MemoryLocationSet):
            continue
        name = alloc.memorylocations[0].name
        if alloc.kind == "ExternalInput":
            if name != partition_name:
                in_names.append(name)
        elif alloc.kind == "ExternalOutput":
            shape = tuple(alloc.tensor_shape)
            dtype = mb.dt.np(alloc.dtype)
            out_names.append(name)
            out_avals.append(jax.core.ShapedArray(shape, dtype))
            zero_outs.append(np.zeros(shape, dtype))
    n_params = len(in_names)
    n_outs = len(out_avals)
    all_in_names = list(in_names) + out_names
    if partition_name is not None:
        all_in_names.append(partition_name)
    donate = tuple(range(n_params, n_params + n_outs))

    def _body(*args):
        operands = list(args)
        if partition_name is not None:
            operands.append(bass2jax.partition_id_tensor())
        outs = bass2jax._bass_exec_p.bind(
            *operands,
            out_avals=tuple(out_avals),
            in_names=tuple(all_in_names),
            out_names=tuple(out_names),
            lowering_input_output_aliases=(),
            sim_require_finite=True,
            sim_require_nnan=True,
            nc=nc,
        )
        return tuple(outs)

    devices = jax.devices()[:n_cores]
    mesh = Mesh(np.asarray(devices), ("core",))
    in_specs = (PartitionSpec("core"),) * (n_params + n_outs)
    out_specs = (PartitionSpec("core"),) * len(out_names)
    sharded = jax.jit(
        shard_map(_body, mesh=mesh, in_specs=in_specs, out_specs=out_specs,
                  check_rep=False),
        donate_argnums=donate, keep_unused=True)

    sharding = jax.sharding.NamedSharding(mesh, PartitionSpec("core"))
    concat_in = [
        jax.device_put(
            np.concatenate([np.asarray(in_maps[c][name])
                            for c in range(n_cores)], axis=0), sharding)
        for name in in_names
    ]

    def fresh_zeros():
        return [
            jax.device_put(
                np.zeros((n_cores * z.shape[0], *z.shape[1:]), z.dtype),
                sharding)
            for z in zero_outs
        ]

    out_arrs = None
    times = []
    for _ in range(max(1, n_reps)):
        zs = fresh_zeros()
        for z in zs:
            z.block_until_ready()
        t0 = time.perf_counter()
        out_arrs = sharded(*concat_in, *zs)
        for o in out_arrs:
            o.block_until_ready()
        times.append(time.perf_counter() - t0)

    _LAST_RESULTS["wall_times_s"] = times
    _LAST_RESULTS["exec_time_ns"] = int(min(times) * 1e9)
    _LAST_RESULTS["nc"] = nc
    _LAST_RESULTS["in_maps"] = in_maps
    return [
        {name: np.asarray(out_arrs[i]).reshape(n_cores, *out_avals[i].shape)[c]
         for i, name in enumerate(out_names)}
        for c in range(n_cores)
    ]


# --------------------------------------------------------------------------
# Host entry point
# --------------------------------------------------------------------------

def _run(cfg, h, src, dst, W, Wb, a, ab, use_sim=False, n_reps=1):
    N, F, H, O, NCORES = cfg["N"], cfg["F"], cfg["H"], cfg["O"], cfg["NCORES"]
    NW = cfg["NW"]
    HO = H * O

    h = np.asarray(h, np.float32)
    src = np.asarray(src).astype(np.int64)
    dst = np.asarray(dst).astype(np.int64)
    W = np.asarray(W, np.float32)
    Wb = np.asarray(Wb, np.float32)
    a = np.asarray(a, np.float32)
    ab = np.asarray(ab, np.float32)

    plan = _plan(cfg, src, dst)
    cfg = dict(cfg, CH=plan["CH"], NS=plan["NS"],
               HALF_ROWS=plan["HALF_ROWS"])
    NS, CH = cfg["NS"], cfg["CH"]
    CW = 2 * CH
    NPC = N // NCORES

    wext, bext = _host_weights(cfg, W, Wb, a, ab)

    # permuted node features, per core
    slot_of, pos_of = plan["slot_of"], plan["pos_of"]
    rows = slot_of.astype(np.int64) * P + pos_of
    in_maps = []
    for c in range(NCORES):
        h_perm = np.zeros((NS, F), np.float32)
        nodes = np.arange(NPC * c, NPC * (c + 1))
        h_perm[rows[nodes]] = h[nodes]
        in_maps.append({
            "hT": np.ascontiguousarray(h_perm.T).astype(BF16),
            "wext": wext.astype(BF16),
            "bext": bext.reshape(1, -1).astype(BF16),
            "ones1": np.ones((1, P), BF16),
            "idxA": _pack_idx(plan["srcidx"][c, 0], CH),
            "idxB": _pack_idx(plan["srcidx"][c, 1], CH),
            "atm": plan["atm"][c],
            "am": plan["am"][c],
        })

    nc = build_gat(cfg)
    nc.compile()

    if use_sim:
        from concourse import bass_interp
        sim = bass_interp.MultiCoreSim(nc, NCORES)
        for c in range(NCORES):
            for k, v in in_maps[c].items():
                sim.cores[c].tensor(k)[:] = v
        sim.simulate()
        outs = [np.array(sim.cores[c].mem_tensor("out_local"))
                for c in range(NCORES)]
    else:
        results = _run_pjrt_timed(nc, in_maps, NCORES, n_reps=n_reps)
        outs = [results[c]["out_local"] for c in range(NCORES)]

    out = np.empty((N, HO), np.float32)
    for c in range(NCORES):
        nodes = np.arange(NPC * c, NPC * (c + 1))
        out[nodes] = outs[c][rows[nodes]]
    return out   # columns already in reference (h, o) order


def _profile_hw(neff_dir=None):
    """Re-run the last-built program under an NRT/NTFF profile and return the
    max per-core HW execution time in ns (the neuron-profile exec time)."""
    import contextlib
    import ctypes
    import shutil
    import tempfile

    nc = _LAST_RESULTS["nc"]
    in_maps = _LAST_RESULTS["in_maps"]
    ncores = len(in_maps)
    if neff_dir is None:
        neff_dir = tempfile.mkdtemp(prefix="gat_ntff_")
    else:
        shutil.rmtree(neff_dir, ignore_errors=True)
        os.makedirs(neff_dir, exist_ok=True)

    so_path = "/opt/axon/libaxon_pjrt.so"
    lib = ctypes.CDLL(so_path)
    assert hasattr(lib, "axon_start_nrt_profile")
    lib.axon_start_nrt_profile.argtypes = [ctypes.POINTER(ctypes.c_int64),
                                           ctypes.c_size_t]
    lib.axon_start_nrt_profile.restype = ctypes.c_int64
    lib.axon_stop_nrt_profile.argtypes = [ctypes.c_char_p]
    lib.axon_stop_nrt_profile.restype = ctypes.c_int64

    @contextlib.contextmanager
    def hook(output_dir, device_ids):
        import jax
        jax.devices()
        ids = (ctypes.c_int64 * len(device_ids))(*device_ids)
        rc = lib.axon_start_nrt_profile(ids, len(device_ids))
        if rc != 0:
            raise RuntimeError(f"axon_start_nrt_profile rc={rc}")
        try:
            yield
        finally:
            n = lib.axon_stop_nrt_profile(str(output_dir).encode())
            if n <= 0:
                raise RuntimeError(f"axon_stop_nrt_profile rc={n}")

    with hook(neff_dir, list(range(ncores))):
        _run_pjrt_timed(nc, in_maps, ncores, n_reps=1)

    from concourse._compat import FishPath
    import gauge.profiler

    profile = gauge.profiler.Profile(
        profile_path=FishPath(neff_dir),
        kernel_dev_mode=True,
        profile_on_exit=False,
        bass_kernel=nc.m,
        offline_processing=True,
        fname="*_body*",
    )
    results = profile.to_perfetto(model_index=tuple(range(ncores)))
    per_core = [r.exec_time_ns for r in results]
    _LAST_RESULTS["hw_exec_time_per_core_ns"] = per_core
    _LAST_RESULTS["hw_exec_time_ns"] = max(per_core)
    return _LAST_RESULTS["hw_exec_time_ns"]


def kernel(h, src, dst, W, Wb, a, ab):
    cfg = dict(FULL_CFG)
    n_reps = int(os.environ.get("GAT_NREPS", "3"))
    use_sim = os.environ.get("GAT_SIM", "0") == "1"
    return _run(cfg, h, src, dst, W, Wb, a, ab, use_sim=use_sim,
                n_reps=n_reps)


# revision 4
# speedup vs baseline: 152.9629x; 152.9629x over previous
"""GAT (graph attention) layer on 8 Trainium2 NeuronCores — v2.

Strategy (dst-partitioned edge parallel, bulk-gather based):
  - Nodes are split into 8 contiguous ranges of 6250; each core's nodes are
    permuted into NW=51 "windows" of <=127 nodes (position 127 = trash slot),
    so a node's permuted row is (core, window, position).
  - Phase A (per core): project the core's node shard with TensorE in bf16:
        [Wh | ed] = h_shard @ [Wmat | A_d] + bias
    where Wh is the per-head projection in (o,h)-interleaved layout (256 cols)
    and ed[h] = <Wh row, a_dst[h]> + c_d + ab (destination attention score).
    Wh rows (512B each) go to a DRAM table that is AllGathered so every core
    holds all 52224 rows; ed rows (padded to 256B) stay in a core-local table.
  - Phase B (per core): edges are grouped by destination window (host-side),
    each window holding <=CH*128 edges per src-half (src row < or >= 26112,
    so bulk dma_gather int16 indices stay in range).  Per window:
      * one bulk dma_gather per src-half: G[128, CW, 256] = Wh[src] rows
      * e_s per edge on DVE: reduce_o(G * a_src)     (a_src broadcast mult)
      * e_d per edge: bulk dma_gather from the local ed table by dst position
      * w = max(exp(s), exp(alpha*s)) = exp(leakyrelu(s))   (ACT + DVE max)
      * one-hot A[e,v] = (iota - dstl == 0) per 128-edge chunk (DVE)
      * numerator+denominator via PSUM-accumulated matmuls:
            agg[v, :] += A_k.T @ [w*G | w]_k
      * out = agg[:, :256] / max(agg[:, 256:], tiny)
  - Host does only index/layout work: window balancing, packing, unscramble.
"""
import os
import sys

sys.path.insert(0, "/opt/trn_rl_repo")

import numpy as np
import ml_dtypes

import concourse.bass as bass
import concourse.bacc as bacc
import concourse.tile as tile
import concourse.mybir as mybir

BF16 = np.dtype(ml_dtypes.bfloat16)
P = 128

FULL_CFG = dict(
    N=50000, F=512, H=8, O=32, ALPHA=0.2, NCORES=8, NW=51,
)

_LAST_RESULTS = {}


# --------------------------------------------------------------------------
# Host-side planning
# --------------------------------------------------------------------------

def _plan(cfg, src, dst):
    """Balance each core's nodes into windows; pack edges into chunk slots."""
    import heapq

    N, NCORES, NW = cfg["N"], cfg["NCORES"], cfg["NW"]
    NPC = N // NCORES              # nodes per core
    NS = NW * P                    # padded rows per core
    E = src.shape[0]

    core_of = (np.arange(N) // NPC).astype(np.int64)
    src_half = (src // NPC) < (NCORES // 2)   # True -> table A

    degA = np.zeros(N, np.int64)
    degB = np.zeros(N, np.int64)
    np.add.at(degA, dst[src_half], 1)
    np.add.at(degB, dst[~src_half], 1)

    slot_of = np.empty(N, np.int32)
    pos_of = np.empty(N, np.int32)
    max_half_edges = 0
    for c in range(NCORES):
        lo, hi = NPC * c, NPC * (c + 1)
        dA, dB = degA[lo:hi], degB[lo:hi]
        order = np.argsort(-(dA + dB), kind="stable")
        la = np.zeros(NW)
        lb = np.zeros(NW)
        cnt = np.zeros(NW, np.int64)
        heap = [(0.0, 0.0, 0, w) for w in range(NW)]
        heapq.heapify(heap)
        for i in order:
            tmp = []
            while True:
                item = heapq.heappop(heap)
                w = item[3]
                if cnt[w] < P - 1:
                    break
                tmp.append(item)
            for t in tmp:
                heapq.heappush(heap, t)
            slot_of[lo + i] = w
            pos_of[lo + i] = cnt[w]
            la[w] += dA[i]
            lb[w] += dB[i]
            cnt[w] += 1
            if cnt[w] < P - 1:
                heapq.heappush(heap, (max(la[w], lb[w]), la[w] + lb[w],
                                      int(cnt[w]), w))
        max_half_edges = max(max_half_edges, int(la.max()), int(lb.max()))

    CH = max(1, -(-max_half_edges // P))   # chunks per half-window
    CW = 2 * CH
    cap = CH * P

    # permuted rows
    perm_row = core_of * NS + slot_of.astype(np.int64) * P + pos_of
    HALF_ROWS = (NCORES // 2) * NS

    # per-core packed arrays
    srcidx = np.zeros((NCORES, 2, P, NW * CH), np.int32)   # [core, half, p, col]
    dstl = np.full((NCORES, P, NW * CW), 127.0, np.float32)

    dst_core = core_of[dst]
    for c in range(NCORES):
        m = dst_core == c
        s_c, d_c = src[m], dst[m]
        half_c = (~src_half[m]).astype(np.int64)   # 0 -> A, 1 -> B
        w_c = slot_of[d_c].astype(np.int64)
        key = w_c * 2 + half_c
        order = np.argsort(key, kind="stable")
        s_c, d_c, half_c, w_c = (s_c[order], d_c[order], half_c[order],
                                 w_c[order])
        key = key[order]
        counts = np.bincount(key, minlength=NW * 2)
        assert counts.max() <= cap, (counts.max(), cap)
        off = np.concatenate([[0], np.cumsum(counts)[:-1]])
        within = np.arange(len(s_c)) - off[key]
        p = within % P
        k = within // P                      # chunk within the half
        # src gather index (relative to its half table)
        g = perm_row[s_c] - half_c * HALF_ROWS
        if len(g):
            assert g.min() >= 0 and g.max() < HALF_ROWS
        srcidx[c, half_c, p, w_c * CH + k] = g
        # one-hot position (dst is local to core c)
        col = w_c * CW + half_c * CH + k
        dstl[c, p, col] = pos_of[d_c]

    # one-hots streamed to the device, per chunk column col and edge slot e:
    #   atm[core][v, col*P + e] = 1 iff dstl[core][e, col] == v   (AT[v, e])
    #   am [core][e, col*P + v] = 1 iff dstl[core][e, col] == v   (A[e, v])
    FP8 = np.dtype(ml_dtypes.float8_e4m3)
    atm = np.zeros((NCORES, P, NW * CW * P), FP8)
    am = np.zeros((NCORES, P, NW * CW * P), FP8)
    cols = np.arange(NW * CW)[None, :].repeat(P, 0)
    e_ix = np.arange(P)[:, None].repeat(NW * CW, 1)
    for c in range(NCORES):
        pos = dstl[c].astype(np.int64)            # [P(e), COLS]
        atm[c, pos.reshape(-1), (cols * P + e_ix).reshape(-1)] = 1.0
        am[c, e_ix.reshape(-1), (cols * P + pos).reshape(-1)] = 1.0

    return dict(CH=CH, NS=NS, HALF_ROWS=HALF_ROWS,
                srcidx=srcidx, dstl=dstl, atm=atm, am=am,
                slot_of=slot_of, pos_of=pos_of)


def _pack_idx(idx_pcol, cols_per_window):
    """[P, COLS] int32 slot-layout -> dma_gather int16 idx layout [128, COLS*8].

    Per window, chunk-columns are gathered in calls of <=8 columns (1024
    indices).  Within a call, flat index i = k*128 + p maps to idx tile
    [i % 16, i // 16]; call data is packed contiguously so the call whose
    first chunk-column is c starts at idx column c*8.
    """
    Ptot, COLS = idx_pcol.shape
    assert Ptot == P
    cpw = cols_per_window
    assert COLS % cpw == 0
    out = np.empty((16, COLS * 8), np.int16)
    pos = 0
    for w in range(COLS // cpw):
        for base in range(0, cpw, 4):
            nb = min(4, cpw - base)
            nidx = nb * P
            flat = idx_pcol[:, w * cpw + base:w * cpw + base + nb].T.reshape(-1)
            out[:, pos:pos + nidx // 16] = flat.reshape(nidx // 16, 16).T
            pos += nidx // 16
    assert pos == COLS * 8
    return np.tile(out, (8, 1))


def _host_weights(cfg, W, Wb, a, ab):
    H, F, O = W.shape
    a_src, a_dst = a[:, :O], a[:, O:]
    Wmat = W.transpose(1, 0, 2).reshape(F, H * O)          # [F, (h,o)]
    A_d = np.einsum("hfo,ho->fh", W, a_dst)
    A_s = np.einsum("hfo,ho->fh", W, a_src)
    wext = np.concatenate([Wmat, A_d, A_s], axis=1)        # [F, 272]
    c_d = (Wb * a_dst).sum(1)
    c_s = (Wb * a_src).sum(1)
    bext = np.concatenate([Wb.reshape(-1), c_d + ab,
                           c_s]).astype(np.float32)
    return wext.astype(np.float32), bext


# --------------------------------------------------------------------------
# Device program
# --------------------------------------------------------------------------

def build_gat(cfg):
    N, F, H, O, NCORES = cfg["N"], cfg["F"], cfg["H"], cfg["O"], cfg["NCORES"]
    NW, CH, NS = cfg["NW"], cfg["CH"], cfg["NS"]
    CW = 2 * CH
    HO = H * O                    # 256
    AD = HO + H                   # 264
    NT = NW                       # phase-A tiles per core
    KT = F // P
    NSG = NS * NCORES
    HALF_ROWS = cfg["HALF_ROWS"]
    RW = 384                      # gather row: Wh(256) | es(8) | pad (768B)
    AD2 = HO + 2 * H              # phase-A psum width: Wh | ed | es

    bf = mybir.dt.bfloat16
    f32 = mybir.dt.float32

    nc = bacc.Bacc("TRN2", target_bir_lowering=False, debug=False,
                   num_devices=NCORES, num_swdge_queues=4)

    hT = nc.dram_tensor("hT", [F, NS], bf, kind="ExternalInput")
    wext = nc.dram_tensor("wext", [F, AD2], bf, kind="ExternalInput")
    bext = nc.dram_tensor("bext", [1, AD2], bf, kind="ExternalInput")
    ones1 = nc.dram_tensor("ones1", [1, P], bf, kind="ExternalInput")
    idxA = nc.dram_tensor("idxA", [P, NW * CH * 8], mybir.dt.int16,
                          kind="ExternalInput")
    idxB = nc.dram_tensor("idxB", [P, NW * CH * 8], mybir.dt.int16,
                          kind="ExternalInput")
    atm = nc.dram_tensor("atm", [P, NW * CW * P], mybir.dt.float8e4,
                         kind="ExternalInput")
    am = nc.dram_tensor("am", [P, NW * CW * P], mybir.dt.float8e4,
                        kind="ExternalInput")

    out_local = nc.dram_tensor("out_local", [NS, HO], bf,
                               kind="ExternalOutput")

    with tile.TileContext(nc) as tc:
        with (
            tc.tile_pool(name="dram", bufs=1, space="DRAM") as dram,
            tc.tile_pool(name="const", bufs=1) as cpool,
        ):
            tbl_local = dram.tile([NS, RW], bf)
            tbl_global = dram.tile([NSG, RW], bf, addr_space="Shared")

            ones_t = cpool.tile([1, P], bf)
            nc.sync.dma_start(out=ones_t[:], in_=ones1[:, :])
            bext_t = cpool.tile([1, AD2], bf)
            nc.sync.dma_start(out=bext_t[:], in_=bext[:, :])
            idxA_t = cpool.tile([P, NW * CH * 8], mybir.dt.int16)
            nc.sync.dma_start(out=idxA_t[:], in_=idxA[:, :])
            idxB_t = cpool.tile([P, NW * CH * 8], mybir.dt.int16)
            nc.sync.dma_start(out=idxB_t[:], in_=idxB[:, :])
            ed_sb = cpool.tile([P, NW * H], bf)
            sE = cpool.tile([P, NW * CW * H], f32)

            # ---------------- Phase A: projection ----------------
            with (
                tc.tile_pool(name="pa_sb", bufs=1) as pa,
                tc.tile_pool(name="pa_ps", bufs=2, space="PSUM") as pa_ps,
            ):
                hT_t = pa.tile([P, KT * NS], bf, tag="hT")
                for kk in range(KT):
                    nc.sync.dma_start(out=hT_t[:, kk * NS:(kk + 1) * NS],
                                      in_=hT[kk * P:(kk + 1) * P, :])
                wext_t = pa.tile([P, KT * AD2], bf, tag="wext")
                for kk in range(KT):
                    nc.sync.dma_start(out=wext_t[:, kk * AD2:(kk + 1) * AD2],
                                      in_=wext[kk * P:(kk + 1) * P, :])

                stage = pa.tile([P, NT * RW], bf, tag="stage")
                nc.vector.memset(stage[:], 0.0)

                for t in range(NT):
                    psA = pa_ps.tile([P, AD2], f32, tag="psA")
                    for kk in range(KT):
                        nc.tensor.matmul(
                            out=psA[:],
                            lhsT=hT_t[:, kk * NS + t * P: kk * NS + (t + 1) * P],
                            rhs=wext_t[:, kk * AD2:(kk + 1) * AD2],
                            start=(kk == 0), stop=False)
                    nc.tensor.matmul(out=psA[:], lhsT=ones_t[:],
                                     rhs=bext_t[:], start=False, stop=True)
                    nc.vector.tensor_copy(
                        out=stage[:, t * RW:t * RW + HO], in_=psA[:, 0:HO])
                    nc.vector.tensor_copy(
                        out=stage[:, t * RW + HO:t * RW + HO + H],
                        in_=psA[:, HO + H:HO + 2 * H])
                    nc.vector.tensor_copy(
                        out=ed_sb[:, t * H:(t + 1) * H],
                        in_=psA[:, HO:HO + H])

                nc.sync.dma_start(
                    out=tbl_local[:].rearrange("(t p) d -> p t d", p=P),
                    in_=stage[:].rearrange("p (t d) -> p t d", t=NT))

            nc.gpsimd.collective_compute(
                "AllGather",
                mybir.AluOpType.bypass,
                replica_groups=[list(range(NCORES))],
                ins=[tbl_local.opt()],
                outs=[tbl_global.opt()],
            )

            # ---------------- e_d expansion (overlaps the AllGather) -------
            with (
                tc.tile_pool(name="at0_sb", bufs=4) as atpool0,
                tc.tile_pool(name="pse0_ps", bufs=4, space="PSUM") as psep0,
            ):
                for w in range(NW):
                    at_t = atpool0.tile([P, CW * P], mybir.dt.float8e4,
                                        tag="at0")
                    nc.sync.dma_start(
                        out=at_t[:], in_=atm[:, w * CW * P:(w + 1) * CW * P])
                    psE = psep0.tile([P, CW * H], f32, tag="psE0")
                    for k in range(CW):
                        nc.tensor.matmul(
                            out=psE[:, k * H:(k + 1) * H],
                            lhsT=at_t[:, k * P:(k + 1) * P],
                            rhs=ed_sb[:, w * H:(w + 1) * H],
                            start=True, stop=True)
                    nc.vector.tensor_copy(
                        out=sE[:, w * CW * H:(w + 1) * CW * H], in_=psE[:])

            # ---------------- Phase B: edges ----------------
            with (
                tc.tile_pool(name="g_sb", bufs=6) as gpool,
                tc.tile_pool(name="s_sb", bufs=3) as spool,
                tc.tile_pool(name="a_sb", bufs=14) as apool,
                tc.tile_pool(name="gp_sb", bufs=3) as gppool,
                tc.tile_pool(name="o_sb", bufs=2) as opool,
                tc.tile_pool(name="agg_ps", bufs=2, space="PSUM") as aggp,
            ):
                viewA = tbl_global[0:HALF_ROWS, :]
                viewB = tbl_global[HALF_ROWS:2 * HALF_ROWS, :]
                for w in range(NW):
                    G = gpool.tile([P, CW * RW], bf, tag="g")
                    ci = 0
                    for half, view, it in ((0, viewA, idxA_t),
                                           (1, viewB, idxB_t)):
                        for base in range(0, CH, 4):
                            nb = min(4, CH - base)
                            nidx = nb * P
                            c0 = half * CH + base
                            nc.gpsimd.dma_gather(
                                G[:, c0 * RW:(c0 + nb) * RW].rearrange(
                                    "p (k e) -> p k e", k=nb),
                                view,
                                it[:, (w * CH + base) * 8:
                                   (w * CH + base) * 8 + nidx // 16],
                                nidx, nidx, RW,
                                queue_num=ci % 4)
                            ci += 1
                    # s = es (gathered) + ed (precomputed during AllGather)
                    s_t = spool.tile([P, CW * H], f32, tag="s")
                    nc.vector.tensor_tensor(
                        out=s_t[:].rearrange("p (k h) -> p k h", k=CW),
                        in0=G[:].rearrange("p (k r) -> p k r",
                                           k=CW)[:, :, HO:HO + H],
                        in1=sE[:, w * CW * H:(w + 1) * CW * H].rearrange(
                            "p (k h) -> p k h", k=CW),
                        op=mybir.AluOpType.add)

                    # w = max(exp(s), exp(alpha*s))
                    w1 = spool.tile([P, CW * H], f32, tag="w1")
                    nc.scalar.activation(out=w1[:], in_=s_t[:],
                                         func=mybir.ActivationFunctionType.Exp)
                    w2 = spool.tile([P, CW * H], f32, tag="w2")
                    nc.scalar.activation(out=w2[:], in_=s_t[:],
                                         func=mybir.ActivationFunctionType.Exp,
                                         scale=float(cfg["ALPHA"]))
                    wv = spool.tile([P, CW * H], f32, tag="wv")
                    nc.vector.tensor_tensor(out=wv[:], in0=w1[:], in1=w2[:],
                                            op=mybir.AluOpType.max)

                    # gp = [w * G | w]
                    gp = gppool.tile([P, CW * AD], bf, tag="gp")
                    gp4 = gp[:].rearrange("p (k d) -> p k d", k=CW)
                    nc.vector.tensor_tensor(
                        out=gp4[:, :, 0:HO].rearrange(
                            "p k (h o) -> p k h o", h=H),
                        in0=G[:].rearrange("p (k r) -> p k r",
                                           k=CW)[:, :, 0:HO].rearrange(
                            "p k (h o) -> p k h o", h=H),
                        in1=wv[:].rearrange("p (k h) -> p k h",
                                            k=CW)[:, :, :, None].to_broadcast(
                            [P, CW, H, O]),
                        op=mybir.AluOpType.mult)
                    nc.scalar.activation(
                        out=gp4[:, :, HO:AD],
                        in_=wv[:].rearrange("p (k h) -> p k h", k=CW),
                        func=mybir.ActivationFunctionType.Copy)

                    # streamed one-hot + aggregation
                    a_t = apool.tile([P, CW * P], mybir.dt.float8e4,
                                     tag="a")
                    nc.sync.dma_start(
                        out=a_t[:], in_=am[:, w * CW * P:(w + 1) * CW * P])
                    agg = aggp.tile([P, AD], f32, tag="agg")
                    for k in range(CW):
                        nc.tensor.matmul(
                            out=agg[:],
                            lhsT=a_t[:, k * P:(k + 1) * P],
                            rhs=gp[:, k * AD:(k + 1) * AD],
                            start=(k == 0), stop=(k == CW - 1))

                    den = opool.tile([P, H], f32, tag="den")
                    nc.vector.tensor_scalar(
                        out=den[:], in0=agg[:, HO:AD],
                        scalar1=1e-30, scalar2=None,
                        op0=mybir.AluOpType.max)
                    rec = opool.tile([P, H], f32, tag="rec")
                    nc.vector.reciprocal(out=rec[:], in_=den[:])
                    o_t = opool.tile([P, HO], bf, tag="o")
                    nc.vector.tensor_tensor(
                        out=o_t[:].rearrange("p (h o) -> p h o", h=H),
                        in0=agg[:, 0:HO].rearrange("p (h o) -> p h o", h=H),
                        in1=rec[:][:, :, None].to_broadcast([P, H, O]),
                        op=mybir.AluOpType.mult)
                    nc.sync.dma_start(
                        out=out_local[w * P:(w + 1) * P, :], in_=o_t[:])

    return nc


# --------------------------------------------------------------------------
# Execution (PJRT path with steady-state timing)
# --------------------------------------------------------------------------

def _run_pjrt_timed(nc, in_maps, n_cores, n_reps=1):
    import time

    import jax
    from jax.sharding import Mesh, PartitionSpec
    from jax.experimental.shard_map import shard_map

    from concourse import bass2jax
    from concourse import mybir as mb

    bass2jax.install_neuronx_cc_hook()

    partition_name = (nc.partition_id_tensor.name
                      if nc.partition_id_tensor else None)

    in_names, out_names, out_avals, zero_outs = [], [], [], []
    for alloc in nc.m.functions[0].allocations:
        if not isinstance(alloc, mb.MemoryLocationSet):
            continue
        name = alloc.memorylocations[0].name
        if alloc.kind == "ExternalInput":
            if name != partition_name:
                in_names.append(name)
        elif alloc.kind == "ExternalOutput":
            shape = tuple(alloc.tensor_shape)
            dtype = mb.dt.np(alloc.dtype)
            out_names.append(name)
            out_avals.append(jax.core.ShapedArray(shape, dtype))
            zero_outs.append(np.zeros(shape, dtype))
    n_params = len(in_names)
    n_outs = len(out_avals)
    all_in_names = list(in_names) + out_names
    if partition_name is not None:
        all_in_names.append(partition_name)
    donate = tuple(range(n_params, n_params + n_outs))

    def _body(*args):
        operands = list(args)
        if partition_name is not None:
            operands.append(bass2jax.partition_id_tensor())
        outs = bass2jax._bass_exec_p.bind(
            *operands,
            out_avals=tuple(out_avals),
            in_names=tuple(all_in_names),
            out_names=tuple(out_names),
            lowering_input_output_aliases=(),
            sim_require_finite=True,
            sim_require_nnan=True,
            nc=nc,
        )
        return tuple(outs)

    devices = jax.devices()[:n_cores]
    mesh = Mesh(np.asarray(devices), ("core",))
    in_specs = (PartitionSpec("core"),) * (n_params + n_outs)
    out_specs = (PartitionSpec("core"),) * len(out_names)
    sharded = jax.jit(
        shard_map(_body, mesh=mesh, in_specs=in_specs, out_specs=out_specs,
                  check_rep=False),
        donate_argnums=donate, keep_unused=True)

    sharding = jax.sharding.NamedSharding(mesh, PartitionSpec("core"))
    concat_in = [
        jax.device_put(
            np.concatenate([np.asarray(in_maps[c][name])
                            for c in range(n_cores)], axis=0), sharding)
        for name in in_names
    ]

    def fresh_zeros():
        return [
            jax.device_put(
                np.zeros((n_cores * z.shape[0], *z.shape[1:]), z.dtype),
                sharding)
            for z in zero_outs
        ]

    out_arrs = None
    times = []
    for _ in range(max(1, n_reps)):
        zs = fresh_zeros()
        for z in zs:
            z.block_until_ready()
        t0 = time.perf_counter()
        out_arrs = sharded(*concat_in, *zs)
        for o in out_arrs:
            o.block_until_ready()
        times.append(time.perf_counter() - t0)

    _LAST_RESULTS["wall_times_s"] = times
    _LAST_RESULTS["exec_time_ns"] = int(min(times) * 1e9)
    _LAST_RESULTS["nc"] = nc
    _LAST_RESULTS["in_maps"] = in_maps
    return [
        {name: np.asarray(out_arrs[i]).reshape(n_cores, *out_avals[i].shape)[c]
         for i, name in enumerate(out_names)}
        for c in range(n_cores)
    ]


# --------------------------------------------------------------------------
# Host entry point
# --------------------------------------------------------------------------

def _run(cfg, h, src, dst, W, Wb, a, ab, use_sim=False, n_reps=1):
    N, F, H, O, NCORES = cfg["N"], cfg["F"], cfg["H"], cfg["O"], cfg["NCORES"]
    NW = cfg["NW"]
    HO = H * O

    h = np.asarray(h, np.float32)
    src = np.asarray(src).astype(np.int64)
    dst = np.asarray(dst).astype(np.int64)
    W = np.asarray(W, np.float32)
    Wb = np.asarray(Wb, np.float32)
    a = np.asarray(a, np.float32)
    ab = np.asarray(ab, np.float32)

    plan = _plan(cfg, src, dst)
    cfg = dict(cfg, CH=plan["CH"], NS=plan["NS"],
               HALF_ROWS=plan["HALF_ROWS"])
    NS, CH = cfg["NS"], cfg["CH"]
    CW = 2 * CH
    NPC = N // NCORES

    wext, bext = _host_weights(cfg, W, Wb, a, ab)

    # permuted node features, per core
    slot_of, pos_of = plan["slot_of"], plan["pos_of"]
    rows = slot_of.astype(np.int64) * P + pos_of
    in_maps = []
    for c in range(NCORES):
        h_perm = np.zeros((NS, F), np.float32)
        nodes = np.arange(NPC * c, NPC * (c + 1))
        h_perm[rows[nodes]] = h[nodes]
        in_maps.append({
            "hT": np.ascontiguousarray(h_perm.T).astype(BF16),
            "wext": wext.astype(BF16),
            "bext": bext.reshape(1, -1).astype(BF16),
            "ones1": np.ones((1, P), BF16),
            "idxA": _pack_idx(plan["srcidx"][c, 0], CH),
            "idxB": _pack_idx(plan["srcidx"][c, 1], CH),
            "atm": plan["atm"][c],
            "am": plan["am"][c],
        })

    nc = build_gat(cfg)
    nc.compile()

    if use_sim:
        from concourse import bass_interp
        sim = bass_interp.MultiCoreSim(nc, NCORES)
        for c in range(NCORES):
            for k, v in in_maps[c].items():
                sim.cores[c].tensor(k)[:] = v
        sim.simulate()
        outs = [np.array(sim.cores[c].mem_tensor("out_local"))
                for c in range(NCORES)]
    else:
        results = _run_pjrt_timed(nc, in_maps, NCORES, n_reps=n_reps)
        outs = [results[c]["out_local"] for c in range(NCORES)]

    out = np.empty((N, HO), np.float32)
    for c in range(NCORES):
        nodes = np.arange(NPC * c, NPC * (c + 1))
        out[nodes] = outs[c][rows[nodes]]
    return out   # columns already in reference (h, o) order


def _profile_hw(neff_dir=None):
    """Re-run the last-built program under an NRT/NTFF profile and return the
    max per-core HW execution time in ns (the neuron-profile exec time)."""
    import contextlib
    import ctypes
    import shutil
    import tempfile

    nc = _LAST_RESULTS["nc"]
    in_maps = _LAST_RESULTS["in_maps"]
    ncores = len(in_maps)
    if neff_dir is None:
        neff_dir = tempfile.mkdtemp(prefix="gat_ntff_")
    else:
        shutil.rmtree(neff_dir, ignore_errors=True)
        os.makedirs(neff_dir, exist_ok=True)

    so_path = "/opt/axon/libaxon_pjrt.so"
    lib = ctypes.CDLL(so_path)
    assert hasattr(lib, "axon_start_nrt_profile")
    lib.axon_start_nrt_profile.argtypes = [ctypes.POINTER(ctypes.c_int64),
                                           ctypes.c_size_t]
    lib.axon_start_nrt_profile.restype = ctypes.c_int64
    lib.axon_stop_nrt_profile.argtypes = [ctypes.c_char_p]
    lib.axon_stop_nrt_profile.restype = ctypes.c_int64

    @contextlib.contextmanager
    def hook(output_dir, device_ids):
        import jax
        jax.devices()
        ids = (ctypes.c_int64 * len(device_ids))(*device_ids)
        rc = lib.axon_start_nrt_profile(ids, len(device_ids))
        if rc != 0:
            raise RuntimeError(f"axon_start_nrt_profile rc={rc}")
        try:
            yield
        finally:
            n = lib.axon_stop_nrt_profile(str(output_dir).encode())
            if n <= 0:
                raise RuntimeError(f"axon_stop_nrt_profile rc={n}")

    with hook(neff_dir, list(range(ncores))):
        _run_pjrt_timed(nc, in_maps, ncores, n_reps=1)

    from concourse._compat import FishPath
    import gauge.profiler

    profile = gauge.profiler.Profile(
        profile_path=FishPath(neff_dir),
        kernel_dev_mode=True,
        profile_on_exit=False,
        bass_kernel=nc.m,
        offline_processing=True,
        fname="*_body*",
    )
    results = profile.to_perfetto(model_index=tuple(range(ncores)))
    per_core = [r.exec_time_ns for r in results]
    _LAST_RESULTS["hw_exec_time_per_core_ns"] = per_core
    _LAST_RESULTS["hw_exec_time_ns"] = max(per_core)
    return _LAST_RESULTS["hw_exec_time_ns"]


def kernel(h, src, dst, W, Wb, a, ab):
    cfg = dict(FULL_CFG)
    n_reps = int(os.environ.get("GAT_NREPS", "3"))
    use_sim = os.environ.get("GAT_SIM", "0") == "1"
    return _run(cfg, h, src, dst, W, Wb, a, ab, use_sim=use_sim,
                n_reps=n_reps)
